# revision 23
# baseline (speedup 1.0000x reference)
"""Trainium2 Bass kernel for nn_CausalSelfAttention_61795989455492.

Sharding (8 cores): core c -> batch b = c//2, head-group hg = c%2 (8 of 16
heads). Each core runs QKV projection (its head slice), rotary, sliding-window
attention with joint prefix softmax, and a partial output projection over its
512 channel columns. Host sums the two partials per batch (pair reduce).

v2 layout notes:
  - all matmul operands bf16 (PSUM accumulation fp32); inputs pre-packed
    p-major on host and DMA'd to SBUF up front (everything is SBUF-resident)
  - rotary shuffle done on PE via a 128x128 permutation matmul (was gpsimd)
  - per 512-col KV chunk, att^T uses a 3-bank PSUM tile with windows
    tj0 [0:384], tj1 [512:896], tj2 [1024:1280], tj3 [1280:1408]; the six
    128-col mask blocks sit at uniform stride 256 so ONE matmul applies them
  - denominator from a ones-column appended to V (row 64 of y_aug);
    normalization via DVE reciprocal_approx_fast + gpsimd partition_broadcast
"""

import sys
from contextlib import ExitStack

import numpy as np

sys.path.insert(0, "/opt/trn_rl_repo")

import ml_dtypes  # noqa: E402
import concourse.bass as bass  # noqa: E402
import concourse.tile as tile_mod  # noqa: E402
from concourse import bacc  # noqa: E402
from concourse import mybir  # noqa: E402

B, T, C, H, D = 4, 512, 1024, 16, 64
S_PREV, PFX, WINDOW = 1536, 256, 256
ROPE_BASE = 10000.0
MASKVAL = -1.0e5
HPC = 8  # heads per core
NCORES = 8

f32 = mybir.dt.float32
f32r = mybir.dt.float32r
bf16 = mybir.dt.bfloat16

# window geometry per 512-col KV chunk, transposed layout:
# s-block tj covers t-run [T0[tj], T0[tj]+TN[tj])
_T0 = [0, 128, 256, 384]
_TN = [384, 384, 256, 128]
# psum column offset of each tj window inside the (128,1536) chunk tile
_POFF = [0, 512, 1024, 1280]
# exp-output column offset of each tj window inside the (128,1152) tile
_EOFF = [0, 384, 768, 1024]


def _emit(nc, tc, io):
    ctx = ExitStack()
    with ctx:
        const = ctx.enter_context(tc.tile_pool(name="const", bufs=1))
        xw = ctx.enter_context(tc.tile_pool(name="xw", bufs=1))
        kvp = ctx.enter_context(tc.tile_pool(name="kvp", bufs=1))
        qkrot = ctx.enter_context(tc.tile_pool(name="qkrot", bufs=1))
        vcur_p = ctx.enter_context(tc.tile_pool(name="vcur", bufs=1))
        ysb = ctx.enter_context(tc.tile_pool(name="ysb", bufs=1))

        # ---------------- startup: all input DMAs ----------------
        sb_w = xw.tile([128, 12, 8, 128], bf16)
        nc.sync.dma_start(out=sb_w[:, 0:1], in_=io["w_p"].ap()[:, 0:1])
        sb_x = xw.tile([128, 8, 512], bf16)
        for i in range(2):
            nc.sync.dma_start(out=sb_x[:, 4 * i:4 * (i + 1), :],
                              in_=io["x_p"].ap()[:, 4 * i:4 * (i + 1), :])
        for m0, m1 in ((1, 4), (4, 8), (8, 12)):
            nc.sync.dma_start(out=sb_w[:, m0:m1],
                              in_=io["w_p"].ap()[:, m0:m1])
        sb_cos = const.tile([128, 512], f32)
        nc.sync.dma_start(out=sb_cos, in_=io["cos2"].ap())
        sb_sin = const.tile([128, 512], f32)
        nc.sync.dma_start(out=sb_sin, in_=io["sin2"].ap())
        sb_P = const.tile([128, 128], bf16)
        nc.sync.dma_start(out=sb_P, in_=io["perm"].ap())
        sb_I = const.tile([128, 128], bf16)
        nc.sync.dma_start(out=sb_I, in_=io["ident"].ap())
        sb_m6 = const.tile([128, 1536], bf16)
        nc.sync.dma_start(out=sb_m6, in_=io["mask6"].ap())
        sb_kts = kvp.tile([128, 4, 1536], bf16)
        for i in range(2):
            nc.sync.dma_start(out=sb_kts[:, 2 * i:2 * (i + 1), :],
                              in_=io["k_p"].ap()[:, 2 * i:2 * (i + 1), :])
        sb_pref = kvp.tile([128, 8, 1024], bf16)
        for i in range(4):
            nc.sync.dma_start(out=sb_pref[:, 2 * i:2 * (i + 1), :],
                              in_=io["pref_p"].ap()[:, 2 * i:2 * (i + 1), :])
        sb_vau = kvp.tile([128, 8, 3, 4, 65], bf16)
        for i in range(2):
            nc.sync.dma_start(out=sb_vau[:, 4 * i:4 * (i + 1)],
                              in_=io["vau_p"].ap()[:, 4 * i:4 * (i + 1)])
        sb_cvn = kvp.tile([128, 8, 2, 65], bf16)
        nc.sync.dma_start(out=sb_cvn, in_=io["cvn_p"].ap())
        sb_wp = kvp.tile([128, 4, 1024], bf16)
        nc.sync.dma_start(out=sb_wp, in_=io["wp_p"].ap())
        ones1 = const.tile([1, 64], bf16)
        nc.sync.dma_start(out=ones1, in_=io["ones_row"].ap())

        q_rot = [qkrot.tile([128, 512], bf16, name=f"qrot{i}", tag=f"qrot{i}") for i in range(4)]
        k_rot = [qkrot.tile([128, 512], bf16, name=f"krot{i}", tag=f"krot{i}") for i in range(4)]
        # current-chunk V per head with ones column: [s=128, h, tb, 65]
        vcur = vcur_p.tile([128, 8, 4, 65], bf16)
        nc.gpsimd.memset(vcur[:, :, :, 64:65], 1.0)
        y_t = [ysb.tile([128, 512], bf16, name=f"ysb{i}", tag=f"ysb{i}") for i in range(4)]

        # ---------------- phase 1: qkv projection + rotary ----------------
        with tc.tile_pool(name="projps", bufs=4, space="PSUM") as projps, \
             tc.tile_pool(name="shps", bufs=2, space="PSUM") as shps_p, \
             tc.tile_pool(name="tmp", bufs=3) as tmp:
            # q^T and k^T: m-tiles 0..7 over qkv rows (q: 0..3, k: 4..7)
            for m in range(8):
                ps = projps.tile([128, 512], f32, name="projps", tag="projps")
                for c in range(8):
                    nc.tensor.matmul(
                        ps,
                        lhsT=sb_w[:, m, c, :],
                        rhs=sb_x[:, c, :],
                        start=(c == 0),
                        stop=(c == 7),
                    )
                # rotary: rot = qk * cos2 + (P^T qk) * sin2
                rot = q_rot[m] if m < 4 else k_rot[m - 4]
                qsb = tmp.tile([128, 512], bf16, name="qsb", tag="qsb")
                nc.scalar.activation(out=qsb, in_=ps,
                                     func=mybir.ActivationFunctionType.Copy)
                shps = shps_p.tile([128, 512], f32, name="shps", tag="shps")
                nc.tensor.matmul(shps, lhsT=sb_P, rhs=qsb, start=True, stop=True)
                ta = tmp.tile([128, 512], bf16, name="ta", tag="ta")
                nc.vector.tensor_mul(ta, ps, sb_cos)
                tb_ = tmp.tile([128, 512], bf16, name="tb", tag="tb")
                nc.vector.tensor_mul(tb_, shps, sb_sin)
                nc.vector.tensor_add(rot, ta, tb_)

            # v natural: t-blocks 0..3 -> scatter into per-head vcur
            for tb in range(4):
                ps = projps.tile([128, 512], f32, name="projps", tag="projps")
                for c in range(8):
                    nc.tensor.matmul(
                        ps,
                        lhsT=sb_x[:, c, tb * 128:(tb + 1) * 128],
                        rhs=sb_w[:, 8:12, c, :],
                        start=(c == 0),
                        stop=(c == 7),
                    )
                nc.scalar.activation(
                    out=vcur[:, :, tb, 0:64],
                    in_=ps.rearrange("p (h d) -> p h d", d=64),
                    func=mybir.ActivationFunctionType.Copy,
                )

        # ---------------- phase 2: attention per head ----------------
        with tc.tile_pool(name="expsb", bufs=2) as exp_p, \
             tc.tile_pool(name="exppref", bufs=2) as expp_p, \
             tc.tile_pool(name="rdn", bufs=2) as rdn_p, \
             tc.tile_pool(name="attps", bufs=2, space="PSUM") as attps_p, \
             tc.tile_pool(name="yaug", bufs=2, space="PSUM") as yaug_p:

            def emit_qk(h, ck, aps):
                hrow = (h % 2) * 64
                hp, mt = h // 2, h // 2
                # mask first: one full-bank-width matmul per psum bank
                # initializes the bank (start=True); sb_m6 holds zeros outside
                # the six 128-col mask blocks. QK then accumulates on top.
                for o0, o1 in ((0, 384), (512, 896), (1024, 1408)):
                    nc.tensor.matmul(
                        aps[:, o0:o1],
                        lhsT=sb_I,
                        rhs=sb_m6[:, o0:o1],
                        start=True,
                        stop=False,
                        skip_group_check=True,
                    )
                for tj in range(4):
                    t0, tn, off = _T0[tj], _TN[tj], _POFF[tj]
                    if ck < 3:
                        kblk = sb_kts[hrow:hrow + 64, hp,
                                      ck * 512 + tj * 128: ck * 512 + (tj + 1) * 128]
                    else:
                        kblk = k_rot[mt][hrow:hrow + 64, tj * 128:(tj + 1) * 128]
                    nc.tensor.matmul(
                        aps[:, off:off + tn],
                        lhsT=kblk,
                        rhs=q_rot[mt][hrow:hrow + 64, t0:t0 + tn],
                        start=False,
                        stop=True,
                        skip_group_check=True,
                    )

            def emit_exp(h, ck, aps):
                ex = exp_p.tile([128, 1152], bf16, name="expsb", tag="expsb")
                nc.scalar.activation(
                    out=ex.rearrange("p (w c) -> p w c", c=384),
                    in_=aps.rearrange("p (w c) -> p w c", c=512)[:, 0:3, 0:384],
                    func=mybir.ActivationFunctionType.Exp,
                    scale=0.125,
                )
                return ex

            def emit_av(h, ck, ex, yps):
                for tj in range(4):
                    t0, tn, eoff = _T0[tj], _TN[tj], _EOFF[tj]
                    if ck < 3:
                        vb = sb_vau[:, h, ck, tj, :]
                    else:
                        vb = vcur[:, h, tj, :]
                    nc.tensor.matmul(
                        yps[0:65, t0:t0 + tn],
                        lhsT=vb,
                        rhs=ex[:, eoff:eoff + tn],
                        start=False,
                        stop=(ck == 3 and tj == 3),
                        skip_group_check=True,
                    )

            def emit_prefix_exp(h):
                expp = expp_p.tile([128, 1024], bf16, name="exppref", tag="exppref")
                nc.scalar.activation(out=expp, in_=sb_pref[:, h, :],
                                     func=mybir.ActivationFunctionType.Exp)
                return expp

            def emit_prefix_av(h, expp):
                yps = yaug_p.tile([128, 512], f32, name="yaug", tag="yaug")
                for pb in range(2):
                    nc.tensor.matmul(
                        yps[0:65, :],
                        lhsT=sb_cvn[:, h, pb, :],
                        rhs=expp[:, pb * 512:(pb + 1) * 512],
                        start=(pb == 0),
                        stop=False,
                        skip_group_check=True,
                    )
                return yps

            def emit_norm_pre(h, yps):
                # reciprocal of the denominator row on DVE (bf16 out)
                rcp_b = rdn_p.tile([1, 512], bf16, name="rcpb", tag="rcpb")
                with nc.allow_low_precision(reason="softmax denom reciprocal"):
                    nc.vector.reciprocal(rcp_b, yps[64:65, :])
                return rcp_b

            def emit_norm_post(h, yps, rcp_b):
                # broadcast 1/denom across 64 partitions into the unused
                # rows 64:128 of the same yps tile (no extra psum needed),
                # then scale and store y^T. Emitted one chunk into the next
                # head so the PE never waits on the DVE reciprocal.
                mt, hrow = h // 2, (h % 2) * 64
                nc.tensor.matmul(yps[64:128, :], lhsT=ones1, rhs=rcp_b,
                                 start=True, stop=True, skip_group_check=True)
                rb = rdn_p.tile([64, 512], f32, name="rb", tag="rb")
                nc.vector.tensor_copy(rb, yps[64:128, :])
                nc.vector.tensor_mul(y_t[mt][hrow:hrow + 64, :], yps[0:64, :], rb)

            # software-pipelined head loop: AV(ck) is emitted after QK(ck+1)
            # -- including across head boundaries -- so the PE never waits on
            # the Scalar exp of the chunk it just scored. The prefix exp of
            # head h+1 is emitted during head h's last chunk; the denominator
            # reciprocal of head h runs during head h+1's first chunks and
            # the normalization lands at h+1's ck2.
            prev = None        # (h, ck, ex, yps) pending AV
            pend_norm = None   # (h, yps, rcp_b)
            expp = emit_prefix_exp(0)
            yps = None
            for h in range(HPC):
                prev_yps = yps
                for ck in range(4):
                    aps = attps_p.tile([128, 1536], f32, name="attps", tag="attps")
                    emit_qk(h, ck, aps)
                    if prev is not None:
                        emit_av(*prev)
                        if prev[1] == 3:
                            # head h-1 fully accumulated: kick its reciprocal
                            rcp_b = emit_norm_pre(h - 1, prev_yps)
                            pend_norm = (h - 1, prev_yps, rcp_b)
                        prev = None
                    if ck == 0:
                        # prefix AV opens this head's yps accumulation; it
                        # sits after qk0/av3 so it also buys slack for the
                        # previous head's reciprocal and y-scale chain
                        yps = emit_prefix_av(h, expp)
                    if ck == 3 and pend_norm is not None:
                        emit_norm_post(*pend_norm)
                        pend_norm = None
                    ex = emit_exp(h, ck, aps)
                    if ck == 3 and h + 1 < HPC:
                        expp = emit_prefix_exp(h + 1)
                    prev = (h, ck, ex, yps)
            # last head: finish the denominator first with ones-row matmuls
            # so the reciprocal overlaps the remaining V-accumulation
            hL, ckL, exL, ypsL = prev
            for tj in range(4):
                t0, tn, eoff = _T0[tj], _TN[tj], _EOFF[tj]
                nc.tensor.matmul(
                    ypsL[64:65, t0:t0 + tn],
                    lhsT=vcur[:, 0, 0, 64:65],
                    rhs=exL[:, eoff:eoff + tn],
                    start=False, stop=(tj == 3), skip_group_check=True,
                )
            rcp_b = emit_norm_pre(HPC - 1, ypsL)
            for tj in range(4):
                t0, tn, eoff = _T0[tj], _TN[tj], _EOFF[tj]
                nc.tensor.matmul(
                    ypsL[0:64, t0:t0 + tn],
                    lhsT=vcur[:, hL, tj, 0:64],
                    rhs=exL[:, eoff:eoff + tn],
                    start=False, stop=(tj == 3), skip_group_check=True,
                )
            emit_norm_post(HPC - 1, ypsL, rcp_b)

        # ---------------- phase 3: output projection (partial) ----------------
        with tc.tile_pool(name="outsb", bufs=3) as out_p, \
             tc.tile_pool(name="cpps", bufs=3, space="PSUM") as cpps_p:
            for tb in range(4):
                for ng in range(2):
                    cps = cpps_p.tile([128, 512], f32, name="cpps", tag="cpps")
                    for ct in range(4):
                        nc.tensor.matmul(
                            cps,
                            lhsT=y_t[ct][:, tb * 128:(tb + 1) * 128],
                            rhs=sb_wp[:, ct, ng * 512:(ng + 1) * 512],
                            start=(ct == 0),
                            stop=(ct == 3),
                        )
                    ob = out_p.tile([128, 512], f32, name="outsb", tag="outsb")
                    nc.scalar.activation(out=ob, in_=cps,
                                         func=mybir.ActivationFunctionType.Copy)
                    nc.sync.dma_start(
                        out=io["out"].ap()[tb * 128:(tb + 1) * 128, ng * 512:(ng + 1) * 512],
                        in_=ob,
                    )


def build_nc():
    nc = bacc.Bacc("TRN2", target_bir_lowering=False, debug=False)
    io = {}
    io["x_p"] = nc.declare_dram_parameter("x_p", [128, 8, 512], bf16, isOutput=False)
    io["w_p"] = nc.declare_dram_parameter("w_p", [128, 12, 8, 128], bf16, isOutput=False)
    io["k_p"] = nc.declare_dram_parameter("k_p", [128, 4, 1536], bf16, isOutput=False)
    io["vau_p"] = nc.declare_dram_parameter("vau_p", [128, 8, 3, 4, 65], bf16, isOutput=False)
    io["pref_p"] = nc.declare_dram_parameter("pref_p", [128, 8, 1024], bf16, isOutput=False)
    io["cvn_p"] = nc.declare_dram_parameter("cvn_p", [128, 8, 2, 65], bf16, isOutput=False)
    io["wp_p"] = nc.declare_dram_parameter("wp_p", [128, 4, 1024], bf16, isOutput=False)
    io["cos2"] = nc.declare_dram_parameter("cos2", [128, 512], f32, isOutput=False)
    io["sin2"] = nc.declare_dram_parameter("sin2", [128, 512], f32, isOutput=False)
    io["perm"] = nc.declare_dram_parameter("perm", [128, 128], bf16, isOutput=False)
    io["ident"] = nc.declare_dram_parameter("ident", [128, 128], bf16, isOutput=False)
    io["mask6"] = nc.declare_dram_parameter("mask6", [128, 1536], bf16, isOutput=False)
    io["ones_row"] = nc.declare_dram_parameter("ones_row", [1, 64], bf16, isOutput=False)
    io["out"] = nc.declare_dram_parameter("out", [512, 1024], f32, isOutput=True)

    with tile_mod.TileContext(nc) as tc:
        _emit(nc, tc, io)
    nc.finalize()
    return nc


def _rotary_tables(start_index):
    half = D // 2
    inv_freq = 1.0 / (ROPE_BASE ** (np.arange(half, dtype=np.float32) / half))
    pos = (float(start_index) + np.arange(T, dtype=np.float32))
    ang = inv_freq[:, None] * pos[None, :]  # (32, 512): [d, t]
    c = np.cos(ang, dtype=np.float32)
    s = np.sin(ang, dtype=np.float32)
    cos2 = np.tile(c, (4, 1))  # (128, 512)
    sin2 = np.tile(np.concatenate([-s, s], axis=0), (2, 1))  # (128, 512)
    return np.ascontiguousarray(cos2), np.ascontiguousarray(sin2)


def _mask_consts():
    ident = np.eye(128, dtype=ml_dtypes.bfloat16)
    i = np.arange(128)
    diag = np.where(i[:, None] > i[None, :], MASKVAL, 0.0).astype(ml_dtypes.bfloat16)
    bound = np.where(i[None, :] > i[:, None], MASKVAL, 0.0).astype(ml_dtypes.bfloat16)
    z = np.zeros((128, 128), dtype=ml_dtypes.bfloat16)
    mask6 = np.concatenate(
        [diag, z, bound, z, diag, z, bound, z, diag, z, diag, z], axis=1)
    # rotary shuffle permutation: sh[i] = qk[sigma(i)], P[p, i] = 1 iff p = sigma(i)
    sig = np.concatenate([i[32:64], i[0:32], i[96:128], i[64:96]])
    perm = np.zeros((128, 128), dtype=ml_dtypes.bfloat16)
    perm[sig, i] = 1.0
    return ident, np.ascontiguousarray(mask6), perm


def _bf(a):
    return np.ascontiguousarray(a.astype(ml_dtypes.bfloat16))


def make_in_maps(x, c_attn_w, c_proj_w, cached_k, cached_v, att_prefix, cache_v, start_index):
    cos2, sin2 = _rotary_tables(np.asarray(start_index).item())
    ident, mask6, perm = _mask_consts()
    in_maps = []
    for core in range(NCORES):
        b, hg = core // 2, core % 2
        hs = slice(hg * HPC, (hg + 1) * HPC)
        r0, r1 = hg * 512, (hg + 1) * 512
        wq = c_attn_w[r0:r1]
        wk = c_attn_w[C + r0:C + r1]
        wv = c_attn_w[2 * C + r0:2 * C + r1]
        w_qkvT = np.concatenate([wq, wk, wv], axis=0).T  # (1024, 1536)
        w_p = w_qkvT.reshape(8, 128, 12, 128).transpose(1, 2, 0, 3)
        x_p = x[b].T.reshape(8, 128, 512).transpose(1, 0, 2)
        # k cache pairs: [pair, 128(2 heads x 64d), 1536] -> p-major
        k_pair = cached_k[b, hs].transpose(0, 2, 1).reshape(4, 128, 1536)
        k_p = k_pair.transpose(1, 0, 2)
        # v cache with ones column: [p, h, ck, tj, 65]
        v6 = cached_v[b, hs].reshape(8, 3, 4, 128, 64).transpose(3, 0, 1, 2, 4)
        vau_p = np.concatenate(
            [v6, np.ones((128, 8, 3, 4, 1), np.float32)], axis=-1)
        # prefix scores transposed: [p, h, 1024]
        p_ = att_prefix[b, hs].transpose(0, 2, 1)  # (8, 256, 512)
        prefT = np.concatenate([p_[:, :128], p_[:, 128:]], axis=2)  # (8,128,1024)
        pref_p = prefT.transpose(1, 0, 2)
        # prefix V with ones column: [p, h, pb, 65]
        cv = cache_v[b, hs].reshape(8, 2, 128, 64).transpose(2, 0, 1, 3)
        cvn_p = np.concatenate(
            [cv, np.ones((128, 8, 2, 1), np.float32)], axis=-1)
        wp_p = c_proj_w[:, r0:r1].T.reshape(4, 128, 1024).transpose(1, 0, 2)
        in_maps.append({
            "x_p": _bf(x_p),
            "w_p": _bf(w_p),
            "k_p": _bf(k_p),
            "vau_p": _bf(vau_p),
            "pref_p": _bf(pref_p),
            "cvn_p": _bf(cvn_p),
            "wp_p": _bf(wp_p),
            "cos2": cos2,
            "sin2": sin2,
            "perm": perm,
            "ident": ident,
            "mask6": mask6,
            "ones_row": np.ones((1, 64), ml_dtypes.bfloat16),
        })
    return in_maps


_NC_CACHE = {}


def kernel(x, c_attn_w, c_proj_w, cached_k, cached_v, att_prefix, cache_v, start_index):
    x = np.asarray(x, dtype=np.float32)
    c_attn_w = np.asarray(c_attn_w, dtype=np.float32)
    c_proj_w = np.asarray(c_proj_w, dtype=np.float32)
    cached_k = np.asarray(cached_k, dtype=np.float32)
    cached_v = np.asarray(cached_v, dtype=np.float32)
    att_prefix = np.asarray(att_prefix, dtype=np.float32)
    cache_v = np.asarray(cache_v, dtype=np.float32)

    if "nc" not in _NC_CACHE:
        _NC_CACHE["nc"] = build_nc()
    nc = _NC_CACHE["nc"]

    in_maps = make_in_maps(x, c_attn_w, c_proj_w, cached_k, cached_v,
                           att_prefix, cache_v, start_index)
    from concourse.bass_utils import run_bass_kernel_spmd
    res = run_bass_kernel_spmd(nc, in_maps, list(range(NCORES)))
    outs = res.results
    y = np.empty((B, T, C), dtype=np.float32)
    for b in range(B):
        y[b] = outs[2 * b]["out"] + outs[2 * b + 1]["out"]
    return y
